# revision 5
# baseline (speedup 1.0000x reference)
"""Embedding-lookup kernel for Trainium2, data-parallel across 8 NeuronCores.

Computes out[b, l] = W[0, i] + W[1, i] + W[2, i] + sum(b), i = input[b, l].

Device strategy (per core, over its row-shard of the input):
  1. Build the fused table wsum[v] = W[0,v] + W[1,v] + W[2,v] + sum(b) in SBUF
     (two vector adds + a matmul broadcast of the bias) and store it to DRAM.
  2. Per-element gather via the stock SWDGE indirect DMA in single-partition-
     destination form: each instruction's dest AP is one partition's
     contiguous row [1, FI, 1]; the descriptor generator then consumes FI
     offsets (one per dest element) from a [128, FI/128] int32 SBUF region in
     partition-fastest order, i.e. a true per-element gather of FI elements
     per instruction.
The host only casts the indices to int32 and reorders them into the
consumption layout; outputs come back in natural order.
"""

import numpy as np
import concourse.bacc as bacc
import concourse.bass as bass
import concourse.mybir as mybir
import concourse.tile as tile

B, L = 16384, 2048
V = 100000
NCORES = 8
P = 128
FV = 782          # table free dim; P * FV = 100096 >= V
VP = P * FV
RB = B // NCORES  # rows per core

TRACE = False     # test harness may flip this; kernel() stores results in LAST
LAST = None


def _build(rows=RB, cols=L, FI=8192, io_bufs=2):
    n_el = rows * cols
    ntiles = n_el // (P * FI)
    assert ntiles * P * FI == n_el
    FC = FI // P
    rows_per_part = FI // cols  # output rows covered by one instruction

    nc = bacc.Bacc("TRN2", target_bir_lowering=False, debug=False,
                   num_devices=NCORES)
    idx_d = nc.dram_tensor("idx", [ntiles, P, FI], mybir.dt.int32,
                           kind="ExternalInput").ap()
    w_d = nc.dram_tensor("w", [3, VP], mybir.dt.float32,
                         kind="ExternalInput").ap()
    b_d = nc.dram_tensor("b", [3, 1], mybir.dt.float32,
                         kind="ExternalInput").ap()
    out_d = nc.dram_tensor("out", [rows, cols], mybir.dt.float32,
                           kind="ExternalOutput").ap()
    wsum_d = nc.dram_tensor("wsum", [VP, 1], mybir.dt.float32).ap()

    with tile.TileContext(nc) as tc:
        with tc.tile_pool(name="setup", bufs=1) as sp, \
             tc.tile_pool(name="psum", bufs=1, space="PSUM") as pp, \
             tc.tile_pool(name="io", bufs=io_bufs) as io:
            # ---- build the fused table ----
            ws = sp.tile([P, FV], mybir.dt.float32, tag="w0")
            w1 = sp.tile([P, FV], mybir.dt.float32, tag="w1")
            w2 = sp.tile([P, FV], mybir.dt.float32, tag="w2")
            nc.sync.dma_start(out=ws[:], in_=w_d[0].rearrange("(p f) -> p f", p=P))
            nc.sync.dma_start(out=w1[:], in_=w_d[1].rearrange("(p f) -> p f", p=P))
            nc.sync.dma_start(out=w2[:], in_=w_d[2].rearrange("(p f) -> p f", p=P))
            b_sb = sp.tile([3, 1], mybir.dt.float32, tag="b")
            nc.sync.dma_start(out=b_sb[:], in_=b_d[:])
            ones = sp.tile([3, P], mybir.dt.float32, tag="ones")
            nc.vector.memset(ones[:], 1.0)
            bsum_ps = pp.tile([P, 1], mybir.dt.float32, space="PSUM")
            nc.tensor.matmul(out=bsum_ps[:], lhsT=ones[:], rhs=b_sb[:],
                             start=True, stop=True)
            bsum = sp.tile([P, 1], mybir.dt.float32, tag="bsum")
            nc.vector.tensor_copy(out=bsum[:], in_=bsum_ps[:])
            nc.vector.tensor_add(ws[:], ws[:], w1[:])
            nc.vector.tensor_add(ws[:], ws[:], w2[:])
            nc.vector.tensor_tensor(out=ws[:], in0=ws[:],
                                    in1=bsum[:, 0:1].to_broadcast([P, FV]),
                                    op=mybir.AluOpType.add)
            nc.sync.dma_start(out=wsum_d.rearrange("(p f) one -> p (f one)", p=P),
                              in_=ws[:])

            # ---- gather loop: per tile, 128 single-partition instructions ----
            for t in range(ntiles):
                it = io.tile([P, FI], mybir.dt.int32, tag="idx")
                vt = io.tile([P, FI], mybir.dt.float32, tag="val")
                nc.sync.dma_start(out=it[:], in_=idx_d[t])
                for p in range(P):
                    nc.gpsimd.indirect_dma_start(
                        out=vt[p:p + 1, :].rearrange("one (f c) -> one f c", c=1),
                        out_offset=None,
                        in_=wsum_d,
                        in_offset=bass.IndirectOffsetOnAxis(
                            ap=it[:, p * FC:(p + 1) * FC], axis=0),
                    )
                r0 = t * P * rows_per_part
                nc.sync.dma_start(
                    out=out_d[r0:r0 + P * rows_per_part, :].rearrange(
                        "(p r) l -> p (r l)", p=P),
                    in_=vt[:])
    nc.compile()
    return nc


def _arrange_idx(idx_flat, FI):
    """Reorder a core's flat int32 index stream into the descriptor-
    consumption layout: instruction (t, p), descriptor m consumes
    region[m % 128, m // 128] of its [128, FI/128] offset region."""
    n = idx_flat.size
    T = n // (P * FI)
    A = idx_flat.reshape(T, P, FI // P, P)           # [t, p, s, cc]
    return np.ascontiguousarray(A.transpose(0, 3, 1, 2)  # [t, cc, p, s]
                                .reshape(T, P, FI))


def _prep_inputs(input, W, b):
    idx = np.ascontiguousarray(np.asarray(input)).astype(np.int32, copy=False)
    Wp = np.zeros((3, VP), np.float32)
    Wp[:, :V] = np.asarray(W, np.float32)
    bb = np.ascontiguousarray(np.asarray(b, np.float32).reshape(3, 1))
    return idx, Wp, bb


def kernel(input, W, b):
    global LAST
    from concourse.bass_utils import run_bass_kernel_spmd

    FI = 8192
    idx, Wp, bb = _prep_inputs(input, W, b)
    nc = _build(FI=FI)
    in_maps = []
    for i in range(NCORES):
        shard = idx[i * RB:(i + 1) * RB].reshape(-1)
        in_maps.append({"idx": _arrange_idx(shard, FI), "w": Wp, "b": bb})
    res = run_bass_kernel_spmd(nc, in_maps, list(range(NCORES)), trace=TRACE)
    LAST = res
    return np.concatenate([res.results[i]["out"] for i in range(NCORES)],
                          axis=0)


# revision 6
# speedup vs baseline: 1.0109x; 1.0109x over previous
"""Sorted-scan embedding-lookup kernel (fast path).

Per core: host sorts the shard's indices. In sorted order the gather output is
a sequence of runs of repeated table values. Device work:
  1. Build fused table wsum[v] in SBUF ([16 x 6272] layout) + store to DRAM.
  2. Scatter each table entry's value to the stream position of its first
     occurrence (stock SWDGE indirect scatter, single-partition-source form:
     ~100K descriptors instead of 4.2M).
  3. Expand runs with one DVE tensor_tensor_scan pass over the stream:
     state = m*state + a  (m=1 inside runs, 0 at run starts).
Host reorders the sorted device output back to natural order.
"""

import numpy as np
import concourse.bacc as bacc
import concourse.bass as bass
import concourse.mybir as mybir
import concourse.tile as tile

B, L = 16384, 2048
V = 100000
NCORES = 8
P = 128
NSEG = 16
FSEG = 6272              # 49 * 128
VP2 = NSEG * FSEG        # 100352
RB = B // NCORES
N = RB * L               # 4_194_304 elements per core
NT = 16                  # scan tiles
CT = N // (P * NT)       # 2048 columns per tile
PSTREAM = N // P         # 32768 positions per partition stream
TRASH = N                # scatter target for unused entries

TRACE = False
LAST = None


def _build():
    FC = FSEG // P  # 49
    nc = bacc.Bacc("TRN2", target_bir_lowering=False, debug=False,
                   num_devices=NCORES)
    w_d = nc.dram_tensor("w", [3, VP2], mybir.dt.float32,
                         kind="ExternalInput").ap()
    b_d = nc.dram_tensor("b", [3, 1], mybir.dt.float32,
                         kind="ExternalInput").ap()
    offw_d = nc.dram_tensor("offw", [P, NSEG * FC], mybir.dt.int32,
                            kind="ExternalInput").ap()
    vg_d = nc.dram_tensor("vg", [P, 2], mybir.dt.int32,
                          kind="ExternalInput").ap()
    ps_d = nc.dram_tensor("ps", [P, 2], mybir.dt.int32,
                          kind="ExternalInput").ap()
    m_d = nc.dram_tensor("m", [P, NT, CT], mybir.dt.float32,
                         kind="ExternalInput").ap()
    outs_d = nc.dram_tensor("outs", [P, NT, CT], mybir.dt.float32,
                            kind="ExternalOutput").ap()
    wsum_d = nc.dram_tensor("wsum", [VP2, 1], mybir.dt.float32).ap()
    asc_d = nc.dram_tensor("asc", [N + P, 1], mybir.dt.float32).ap()

    asc_tiles = asc_d[0:N, :].rearrange("(p t c) one -> p t (c one)", p=P, t=NT)

    with tile.TileContext(nc) as tc:
        with tc.tile_pool(name="setup", bufs=1) as sp, \
             tc.tile_pool(name="psum", bufs=1, space="PSUM") as pp, \
             tc.tile_pool(name="io", bufs=3) as io:
            # ---- fused table in [NSEG, FSEG] layout ----
            ws = sp.tile([NSEG, FSEG], mybir.dt.float32, tag="w0")
            w1 = sp.tile([NSEG, FSEG], mybir.dt.float32, tag="w1")
            w2 = sp.tile([NSEG, FSEG], mybir.dt.float32, tag="w2")
            nc.sync.dma_start(out=ws[:], in_=w_d[0].rearrange("(s f) -> s f", s=NSEG))
            nc.sync.dma_start(out=w1[:], in_=w_d[1].rearrange("(s f) -> s f", s=NSEG))
            nc.sync.dma_start(out=w2[:], in_=w_d[2].rearrange("(s f) -> s f", s=NSEG))
            b_sb = sp.tile([3, 1], mybir.dt.float32, tag="b")
            nc.sync.dma_start(out=b_sb[:], in_=b_d[:])
            ones = sp.tile([3, NSEG], mybir.dt.float32, tag="ones")
            nc.vector.memset(ones[:], 1.0)
            bsum_ps = pp.tile([NSEG, 1], mybir.dt.float32, space="PSUM")
            nc.tensor.matmul(out=bsum_ps[:], lhsT=ones[:], rhs=b_sb[:],
                             start=True, stop=True)
            bsum = sp.tile([NSEG, 1], mybir.dt.float32, tag="bsum")
            nc.vector.tensor_copy(out=bsum[:], in_=bsum_ps[:])
            nc.vector.tensor_add(ws[:], ws[:], w1[:])
            nc.vector.tensor_add(ws[:], ws[:], w2[:])
            nc.vector.tensor_tensor(out=ws[:], in0=ws[:],
                                    in1=bsum[:, 0:1].to_broadcast([NSEG, FSEG]),
                                    op=mybir.AluOpType.add)
            nc.sync.dma_start(
                out=wsum_d.rearrange("(s f) one -> s (f one)", s=NSEG),
                in_=ws[:])

            # ---- zero the a-stream scratch ----
            zt = sp.tile([P, CT], mybir.dt.float32, tag="zero")
            nc.vector.memset(zt[:], 0.0)
            for t in range(NT):
                nc.sync.dma_start(out=asc_tiles[:, t, :], in_=zt[:])

            # ---- offsets to SBUF ----
            offw = sp.tile([P, NSEG * FC], mybir.dt.int32, tag="offw")
            nc.sync.dma_start(out=offw[:], in_=offw_d[:])
            vg = sp.tile([P, 2], mybir.dt.int32, tag="vg")
            nc.sync.dma_start(out=vg[:], in_=vg_d[:])
            ps = sp.tile([P, 2], mybir.dt.int32, tag="ps")
            nc.sync.dma_start(out=ps[:], in_=ps_d[:])

            # ---- crossing values: gather 128 entries, scatter to stream ----
            vx = sp.tile([1, P], mybir.dt.float32, tag="vx")
            nc.gpsimd.indirect_dma_start(
                out=vx[0:1, :].rearrange("one (f c) -> one f c", c=1),
                out_offset=None,
                in_=wsum_d,
                in_offset=bass.IndirectOffsetOnAxis(ap=vg[:, 0:1], axis=0),
            )
            # ---- main scatters: one per table segment ----
            for s in range(NSEG):
                nc.gpsimd.indirect_dma_start(
                    out=asc_d,
                    out_offset=bass.IndirectOffsetOnAxis(
                        ap=offw[:, s * FC:(s + 1) * FC], axis=0),
                    in_=ws[s:s + 1, :].rearrange("one (f c) -> one f c", c=1),
                    in_offset=None,
                )
            nc.gpsimd.indirect_dma_start(
                out=asc_d,
                out_offset=bass.IndirectOffsetOnAxis(ap=ps[:, 0:1], axis=0),
                in_=vx[0:1, :].rearrange("one (f c) -> one f c", c=1),
                in_offset=None,
            )

            # ---- scan tiles ----
            lc_prev = None
            for t in range(NT):
                at = io.tile([P, CT], mybir.dt.float32, tag="a")
                mt = io.tile([P, CT], mybir.dt.float32, tag="m")
                st = io.tile([P, CT], mybir.dt.float32, tag="s")
                nc.sync.dma_start(out=at[:], in_=asc_tiles[:, t, :])
                nc.sync.dma_start(out=mt[:], in_=m_d[:, t, :])
                nc.vector.tensor_tensor_scan(
                    out=st[:], data0=mt[:], data1=at[:],
                    initial=(0.0 if t == 0 else lc_prev[:, 0:1]),
                    op0=mybir.AluOpType.mult, op1=mybir.AluOpType.add)
                lc = sp.tile([P, 1], mybir.dt.float32, tag=f"lc{t}")
                nc.vector.tensor_copy(out=lc[:], in_=st[:, CT - 1:CT])
                lc_prev = lc
                nc.sync.dma_start(out=outs_d[:, t, :], in_=st[:])
    nc.compile()
    return nc


def _host_prep(flat_idx):
    """Per-core host prep. Returns (order, offw, vg, ps, m)."""
    order = np.argsort(flat_idx, kind="stable")
    sv = flat_idx[order]
    runstart = np.empty(N, bool)
    runstart[0] = True
    np.not_equal(sv[1:], sv[:-1], out=runstart[1:])
    # first-occurrence stream position per table entry
    s_off = np.full(VP2, TRASH, np.int32)
    rs_pos = np.flatnonzero(runstart)
    s_off[sv[rs_pos]] = rs_pos.astype(np.int32)
    # scatter offset layout: region[cc, col] = s_off[s*FSEG + col*128 + cc]
    FC = FSEG // P
    offw = np.ascontiguousarray(
        s_off.reshape(NSEG, FC, P).transpose(2, 0, 1).reshape(P, NSEG * FC))
    # partition-stream crossings
    pstarts = np.arange(P) * PSTREAM
    vcross = sv[pstarts].astype(np.int32)
    pcross = pstarts.astype(np.int32)
    pcross_eff = pcross.copy()
    pcross_eff[0] = TRASH  # p=0 handled by natural run start
    vg = np.zeros((P, 2), np.int32)
    vg[:, 0] = vcross
    vg[:, 1] = 0          # second column read but value unused (lands in vx[:,?])
    ps = np.zeros((P, 2), np.int32)
    ps[:, 0] = pcross_eff
    ps[:, 1] = TRASH
    # carry mask
    m = np.ones(N, np.float32)
    m[rs_pos] = 0.0
    m[pstarts] = 0.0
    return order, offw, vg, ps, np.ascontiguousarray(m.reshape(P, NT, CT))


def _prep_wb(W, b):
    Wp = np.zeros((3, VP2), np.float32)
    Wp[:, :V] = np.asarray(W, np.float32)
    bb = np.ascontiguousarray(np.asarray(b, np.float32).reshape(3, 1))
    return Wp, bb


def kernel(input, W, b):
    global LAST
    from concourse.bass_utils import run_bass_kernel_spmd

    idx = np.ascontiguousarray(np.asarray(input)).astype(np.int32, copy=False)
    Wp, bb = _prep_wb(W, b)
    nc = _build()
    in_maps = []
    orders = []
    for i in range(NCORES):
        flat = idx[i * RB:(i + 1) * RB].reshape(-1)
        order, offw, vg, ps, m = _host_prep(flat)
        orders.append(order)
        in_maps.append({"w": Wp, "b": bb, "offw": offw, "vg": vg,
                       "ps": ps, "m": m})
    res = run_bass_kernel_spmd(nc, in_maps, list(range(NCORES)), trace=TRACE)
    LAST = res
    out = np.empty((B, L), np.float32)
    for i in range(NCORES):
        sorted_out = res.results[i]["outs"].reshape(-1)
        shard = np.empty(N, np.float32)
        shard[orders[i]] = sorted_out
        out[i * RB:(i + 1) * RB] = shard.reshape(RB, L)
    return out
